# revision 1
# baseline (speedup 1.0000x reference)
"""GQA kernel for Trainium2, 8 NeuronCores.

Sharding: core c = b*4 + g  handles batch b, kv-head g (4 query heads).
Each core computes:
  Q_g^T = Wq_g @ x_q^T        [4 heads][128, S]   (scale 1/sqrt(D) folded in)
  K_g^T = Wk_g @ x_k^T        [128, S]
  V_g   = (x_v @ Wv_g.T)      [S, 128]  (via V^T then PE transpose)
  S^T   = K tile @ Q^T        [k,q] orientation -> +mask (diag) -> exp
  o^T  += V[kt] matmul P~^T   (PSUM accum), l += ones^T P~^T
  o_norm^T = o^T * recip(bcast l)
  partial = o_norm @ Wo_g.T   [S, E]
Host sums the 4 partials per batch.

Matmuls run in bf16 (fp32 PSUM accumulation): 4-byte dtypes serialize
LDWEIGHTS with the matmul (~191ns per 128x128 load, no FWL/prefetch),
which was ~37% of the kernel span in fp32r. l is broadcast across
partitions with a K=1 matmul so the reciprocal runs at full DVE lane
width ([128,512] not [1,512]).
"""

import sys

import numpy as np

for _p in ("/opt/trn_rl_repo",):
    if _p not in sys.path:
        sys.path.insert(0, _p)

import ml_dtypes

import concourse.bass as bass
import concourse.mybir as mybir
from concourse import bacc
from concourse.bass_utils import run_bass_kernel_spmd
from concourse.masks import make_identity
from concourse.tile import TileContext

B, S, E = 2, 2048, 2048
H, HKV = 16, 4
D = E // H  # 128
G = H // HKV  # 4 query heads per kv head
GD = G * D  # 512
NCORES = B * HKV  # 8
SC = 512  # s/q chunk width (free dim of matmuls)
NSC = S // SC  # 4
NET = E // 128  # 16 e-tiles (contraction)
NKT = S // 128  # 16 k-tiles
SCALE = 1.0 / float(np.sqrt(D))

F32 = mybir.dt.float32
BF16 = mybir.dt.bfloat16
F32R = mybir.dt.float32r
AF = mybir.ActivationFunctionType
NPBF = np.dtype(ml_dtypes.bfloat16)


def build_nc():
    nc = bacc.Bacc()
    xq = nc.declare_dram_parameter("xq", [E, S], BF16, isOutput=False)  # query[b].T
    xk = nc.declare_dram_parameter("xk", [E, S], BF16, isOutput=False)  # key[b].T
    xv = nc.declare_dram_parameter("xv", [E, S], BF16, isOutput=False)  # value[b].T
    wq = nc.declare_dram_parameter("wq", [E, GD], BF16, isOutput=False)
    wk = nc.declare_dram_parameter("wk", [E, D], BF16, isOutput=False)
    wv = nc.declare_dram_parameter("wv", [E, D], BF16, isOutput=False)
    wo = nc.declare_dram_parameter("wo", [GD, E], BF16, isOutput=False)
    msk = nc.declare_dram_parameter("msk", [4 * 128, SC], F32, isOutput=False)
    out = nc.declare_dram_parameter("out", [S, E], F32, isOutput=True)

    with TileContext(nc) as tc:
        with (
            tc.tile_pool(name="singles", bufs=1) as singles,
            tc.tile_pool(name="xt", bufs=24) as xtp,
            tc.tile_pool(name="pexp", bufs=4) as pexp,
            tc.tile_pool(name="small", bufs=2) as small,
            tc.tile_pool(name="ob", bufs=3) as obp,
            tc.tile_pool(name="acc", bufs=4, space="PSUM") as acc,
            tc.tile_pool(name="ops", bufs=2, space="PSUM") as ops,
            tc.tile_pool(name="lps", bufs=1, space="PSUM") as lps,
            tc.tile_pool(name="trp", bufs=1, space="PSUM") as trp,
            tc.tile_pool(name="drp", bufs=2, space="DRAM") as drp,
        ):
            # ---- constants / weights resident in SBUF ----
            wq_sb = singles.tile([128, NET, GD], BF16)  # 16KB/p
            wk_sb = singles.tile([128, NET, D], BF16)  # 4KB/p
            wv_sb = singles.tile([128, NET, D], BF16)  # 4KB/p
            wo_sb = singles.tile([128, G, E], BF16)  # 16KB/p
            mask_sb = singles.tile([128, 4, SC], F32)  # 8KB/p
            ident_f = singles.tile([128, 128], F32)
            ident = singles.tile([128, 128], BF16)
            ones_f = singles.tile([128, 1], F32)
            ones = singles.tile([128, 1], BF16)
            qT = singles.tile([128, G, S], BF16)  # 16KB/p
            kT = singles.tile([128, S], BF16)  # 4KB/p
            v_sb = singles.tile([128, NKT, D], BF16)  # 4KB/p
            onrm = singles.tile([128, G, S], BF16)  # 16KB/p
            o_unn = singles.tile([128, G, S], F32)  # 32KB/p

            make_identity(nc, ident_f)
            nc.scalar.activation(out=ident[:], in_=ident_f[:], func=AF.Copy)
            nc.vector.memset(ones_f, 1.0)
            nc.scalar.activation(out=ones[:], in_=ones_f[:], func=AF.Copy)
            for t in range(NET):
                nc.sync.dma_start(
                    out=wq_sb[:, t, :], in_=wq[t * 128 : (t + 1) * 128, :]
                )
                nc.sync.dma_start(out=wk_sb[:, t, :], in_=wk[t * 128 : (t + 1) * 128, :])
                nc.sync.dma_start(out=wv_sb[:, t, :], in_=wv[t * 128 : (t + 1) * 128, :])
            for h in range(G):
                nc.sync.dma_start(
                    out=wo_sb[:, h, :], in_=wo[h * 128 : (h + 1) * 128, :]
                )
            for j in range(4):
                nc.sync.dma_start(
                    out=mask_sb[:, j, :], in_=msk[j * 128 : (j + 1) * 128, :]
                )

            # ---- phase 1: projections ----
            for sc in range(NSC):
                ssl = slice(sc * SC, (sc + 1) * SC)
                # Q^T: 4 heads
                xts = []
                for t in range(NET):
                    xt = xtp.tile([128, SC], BF16, tag="xt")
                    nc.sync.dma_start(out=xt, in_=xq[t * 128 : (t + 1) * 128, ssl])
                    xts.append(xt)
                for h in range(G):
                    ps = acc.tile([128, SC], F32, tag="acc")
                    for t in range(NET):
                        nc.tensor.matmul(
                            ps[:],
                            lhsT=wq_sb[:, t, h * D : (h + 1) * D],
                            rhs=xts[t][:],
                            start=(t == 0),
                            stop=(t == NET - 1),
                        )
                    # fold softmax scale into Q
                    nc.scalar.activation(
                        out=qT[:, h, ssl], in_=ps[:], func=AF.Copy, scale=SCALE
                    )
                # K^T
                xts = []
                for t in range(NET):
                    xt = xtp.tile([128, SC], BF16, tag="xt")
                    nc.sync.dma_start(out=xt, in_=xk[t * 128 : (t + 1) * 128, ssl])
                    xts.append(xt)
                ps = acc.tile([128, SC], F32, tag="acc")
                for t in range(NET):
                    nc.tensor.matmul(
                        ps[:],
                        lhsT=wk_sb[:, t, :],
                        rhs=xts[t][:],
                        start=(t == 0),
                        stop=(t == NET - 1),
                    )
                nc.vector.tensor_copy(out=kT[:, ssl], in_=ps[:])
                # V^T then transpose to V [s, d]
                xts = []
                for t in range(NET):
                    xt = xtp.tile([128, SC], BF16, tag="xt")
                    nc.sync.dma_start(out=xt, in_=xv[t * 128 : (t + 1) * 128, ssl])
                    xts.append(xt)
                ps = acc.tile([128, SC], F32, tag="acc")
                for t in range(NET):
                    nc.tensor.matmul(
                        ps[:],
                        lhsT=wv_sb[:, t, :],
                        rhs=xts[t][:],
                        start=(t == 0),
                        stop=(t == NET - 1),
                    )
                vt_tmp = small.tile([128, SC], BF16, tag="vt")
                nc.scalar.activation(out=vt_tmp[:], in_=ps[:], func=AF.Copy)
                for i in range(SC // 128):
                    tp = trp.tile([128, 128], BF16, tag="tr")
                    nc.tensor.transpose(
                        tp[:], vt_tmp[:, i * 128 : (i + 1) * 128], ident[:]
                    )
                    nc.vector.tensor_copy(out=v_sb[:, sc * 4 + i, :], in_=tp[:])

            # ---- phase 2+3: attention, outproj interleaved per q-chunk ----
            for qc in range(NSC):
                for h in range(G):
                    qsl = slice(qc * SC, (qc + 1) * SC)
                    nkt = (qc + 1) * (SC // 128)  # causal: k tiles 0..nkt-1
                    o_ps = ops.tile([128, SC], F32, tag="o")
                    l_ps = lps.tile([1, SC], F32, tag="l")
                    for kt in range(nkt):
                        s_ps = acc.tile([128, SC], F32, tag="acc")
                        nc.tensor.matmul(
                            s_ps[:],
                            lhsT=kT[:, kt * 128 : (kt + 1) * 128],
                            rhs=qT[:, h, qsl],
                            start=True,
                            stop=True,
                        )
                        if kt >= nkt - 4:
                            j = kt - 4 * qc
                            nc.vector.tensor_add(s_ps[:], s_ps[:], mask_sb[:, j, :])
                        p_sb = pexp.tile([128, SC], BF16, tag="p")
                        nc.scalar.activation(out=p_sb[:], in_=s_ps[:], func=AF.Exp)
                        nc.tensor.matmul(
                            o_ps[:],
                            lhsT=v_sb[:, kt, :],
                            rhs=p_sb[:],
                            start=(kt == 0),
                            stop=(kt == nkt - 1),
                        )
                        nc.tensor.matmul(
                            l_ps[:],
                            lhsT=ones[:],
                            rhs=p_sb[:],
                            start=(kt == 0),
                            stop=(kt == nkt - 1),
                        )
                    # l broadcast across partitions via K=1 matmul, then
                    # reciprocal at full lane width and normalize.
                    nc.scalar.activation(
                        out=o_unn[:, h, qsl], in_=o_ps[:], func=AF.Copy
                    )
                    l_sb = small.tile([1, SC], F32, tag="lsb")
                    nc.scalar.activation(out=l_sb[:], in_=l_ps[:], func=AF.Copy)
                    l_dr = drp.tile([1, SC], F32, tag="ldr")
                    nc.sync.dma_start(out=l_dr[:], in_=l_sb[:])
                    lb = small.tile([128, SC], F32, tag="lb")
                    l_bc = bass.AP(
                        tensor=l_dr[:].tensor,
                        offset=l_dr[:].offset,
                        ap=[[0, 128]] + list(l_dr[:].ap[1:]),
                    )
                    nc.sync.dma_start(out=lb[:], in_=l_bc)
                    rb = small.tile([128, SC], F32, tag="rb")
                    nc.vector.reciprocal(out=rb[:], in_=lb[:])
                    nc.vector.tensor_mul(
                        onrm[:, h, qsl], o_unn[:, h, qsl], rb[:]
                    )

                # output projection for this q-chunk's 4 s-tiles
                for sti in range(SC // 128):
                    st = qc * (SC // 128) + sti
                    stl = slice(st * 128, (st + 1) * 128)
                    for ec in range(E // SC):
                        esl = slice(ec * SC, (ec + 1) * SC)
                        ps = acc.tile([128, SC], F32, tag="acc")
                        for h in range(G):
                            nc.tensor.matmul(
                                ps[:],
                                lhsT=onrm[:, h, stl],
                                rhs=wo_sb[:, h, esl],
                                start=(h == 0),
                                stop=(h == G - 1),
                            )
                        ob = obp.tile([128, SC], F32, tag="ob")
                        nc.scalar.activation(out=ob[:], in_=ps[:], func=AF.Copy)
                        nc.sync.dma_start(out=out[stl, esl], in_=ob[:])
    nc.compile()
    return nc


_NC_CACHE = None


def _get_nc():
    global _NC_CACHE
    if _NC_CACHE is None:
        _NC_CACHE = build_nc()
    return _NC_CACHE


def _prep_in_maps(query, key, value, attn_mask, Wq, Wk, Wv, Wo):
    query = np.asarray(query, dtype=np.float32)
    key = np.asarray(key, dtype=np.float32)
    value = np.asarray(value, dtype=np.float32)
    Wq = np.asarray(Wq, dtype=np.float32)
    Wk = np.asarray(Wk, dtype=np.float32)
    Wv = np.asarray(Wv, dtype=np.float32)
    Wo = np.asarray(Wo, dtype=np.float32)
    am = np.asarray(attn_mask)

    xqT = [np.ascontiguousarray(query[b].T).astype(NPBF) for b in range(B)]
    xkT = [np.ascontiguousarray(key[b].T).astype(NPBF) for b in range(B)]
    xvT = [np.ascontiguousarray(value[b].T).astype(NPBF) for b in range(B)]

    # 4 diagonal mask tiles [128, SC]: tile j covers k in [j*128,(j+1)*128)
    # relative to the q-chunk start; additive -1e9 on masked entries.
    m0 = np.asarray(am[0, 0, :SC, :SC], dtype=np.float32)  # [q, k] for chunk 0
    msk_tiles = np.zeros((4 * 128, SC), dtype=np.float32)
    for j in range(4):
        msk_tiles[j * 128 : (j + 1) * 128, :] = (
            m0[:, j * 128 : (j + 1) * 128].T - 1.0
        ) * 1e9
    in_maps = []
    for b in range(B):
        for g in range(HKV):
            in_maps.append(
                {
                    "xq": xqT[b],
                    "xk": xkT[b],
                    "xv": xvT[b],
                    "wq": np.ascontiguousarray(
                        Wq[g * GD : (g + 1) * GD, :].T
                    ).astype(NPBF),
                    "wk": np.ascontiguousarray(
                        Wk[g * D : (g + 1) * D, :].T
                    ).astype(NPBF),
                    "wv": np.ascontiguousarray(
                        Wv[g * D : (g + 1) * D, :].T
                    ).astype(NPBF),
                    "wo": np.ascontiguousarray(
                        Wo[:, g * GD : (g + 1) * GD].T
                    ).astype(NPBF),
                    "msk": msk_tiles,
                }
            )
    return in_maps


def _run(inputs, trace=False, **kw):
    nc = _get_nc()
    in_maps = _prep_in_maps(**inputs)
    res = run_bass_kernel_spmd(
        nc, in_maps, list(range(NCORES)), trace=trace, **kw
    )
    outs = [np.asarray(r["out"]) for r in res.results]
    full = np.empty((B, S, E), dtype=np.float32)
    for b in range(B):
        acc = outs[b * HKV].astype(np.float32)
        for g in range(1, HKV):
            acc = acc + outs[b * HKV + g]
        full[b] = acc
    return full, res


def kernel(**inputs):
    full, _ = _run(inputs, trace=False)
    return full



# revision 3
# speedup vs baseline: 1.1177x; 1.1177x over previous
"""GQA kernel for Trainium2, 8 NeuronCores.

Sharding: core c = b*4 + g  handles batch b, kv-head g (4 query heads).
Each core computes (bf16 matmuls, fp32 PSUM):
  Q_g^T = Wq_g @ x_q^T   [4 heads][128, S]  (1/sqrt(D) folded into Wq host-side)
  K_g^T = Wk_g @ x_k^T   [128, S]
  V_g   = via V^T then DMA-xbar transpose   [S, 128]
  per q-chunk (512) ascending k-tiles with causal diagonal subranges:
    S^T = K_kt @ Q^T  (only q >= k columns on diagonal tiles)
    mask add only on the single 128x128 triangular diagonal block
    p = exp(S^T) -> PV accumulate (o^T), l row-sums via two concurrent
    col-tiled M=1 matmuls (partitions 0/32 of one PSUM bank)
  o_norm^T = o^T * bcast(1/l)   (recip_approx_fast on [1,512], PE bcast)
  out_partial = o_norm @ Wo_g.T stored bf16; host sums 4 partials/batch.
"""

import sys

import numpy as np

for _p in ("/opt/trn_rl_repo",):
    if _p not in sys.path:
        sys.path.insert(0, _p)

import ml_dtypes

import concourse.bass as bass
import concourse.mybir as mybir
from concourse import bacc
from concourse.bass_utils import run_bass_kernel_spmd
from concourse.tile import TileContext

B, S, E = 2, 2048, 2048
H, HKV = 16, 4
D = E // H  # 128
G = H // HKV  # 4 query heads per kv head
GD = G * D  # 512
NCORES = B * HKV  # 8
SC = 512  # q-chunk width
NSC = S // SC  # 4
NET = E // 128  # 16 e-tiles (contraction)
NKT = S // 128  # 16 k-tiles
SCALE = 1.0 / float(np.sqrt(D))

F32 = mybir.dt.float32
BF16 = mybir.dt.bfloat16
AF = mybir.ActivationFunctionType
NPBF = np.dtype(ml_dtypes.bfloat16)


def build_nc():
    nc = bacc.Bacc()
    xq = nc.declare_dram_parameter("xq", [E, S], BF16, isOutput=False)  # query[b].T
    xk = nc.declare_dram_parameter("xk", [E, S], BF16, isOutput=False)  # key[b].T
    xv = nc.declare_dram_parameter("xv", [E, S], BF16, isOutput=False)  # value[b].T
    wq = nc.declare_dram_parameter("wq", [E, GD], BF16, isOutput=False)
    wk = nc.declare_dram_parameter("wk", [E, D], BF16, isOutput=False)
    wv = nc.declare_dram_parameter("wv", [E, D], BF16, isOutput=False)
    wo = nc.declare_dram_parameter("wo", [GD, E], BF16, isOutput=False)
    msk = nc.declare_dram_parameter("msk", [128, 128], F32, isOutput=False)
    out = nc.declare_dram_parameter("out", [S, E], BF16, isOutput=True)

    def tiled_ap(dram, rowstride, ntile, ncol):
        # [128, ntile, ncol] view of a row-major DRAM [ntile*128, ncol] tensor
        base = dram[:, :]
        return bass.AP(
            tensor=base.tensor,
            offset=base.offset,
            ap=[[rowstride, 128], [128 * rowstride, ntile], [1, ncol]],
        )

    def x_chunk_ap(dram, tg, c):
        # e-tiles 4*tg..4*tg+3, s-columns [c*SC, (c+1)*SC)
        base = dram[:, :]
        return bass.AP(
            tensor=base.tensor,
            offset=base.offset + tg * 4 * 128 * S + c * SC,
            ap=[[S, 128], [128 * S, 4], [1, SC]],
        )

    with TileContext(nc) as tc:
        with (
            tc.tile_pool(name="singles", bufs=1) as singles,
            tc.tile_pool(name="xs", bufs=6) as xsp,
            tc.tile_pool(name="pexp", bufs=4) as pexp,
            tc.tile_pool(name="vtp", bufs=2) as vtp,
            tc.tile_pool(name="rpp", bufs=2) as rpp,
            tc.tile_pool(name="rbp", bufs=2) as rbp,
            tc.tile_pool(name="obp", bufs=2) as obp,
            tc.tile_pool(name="pacc", bufs=3, space="PSUM") as pacc,
            tc.tile_pool(name="sacc", bufs=2, space="PSUM") as sacc,
            tc.tile_pool(name="ops", bufs=2, space="PSUM") as ops,
            tc.tile_pool(name="lps", bufs=1, space="PSUM") as lps,
        ):
            # ---- resident weights / constants ----
            wq_sb = singles.tile([128, NET, GD], BF16)
            wk_sb = singles.tile([128, NET, D], BF16)
            wv_sb = singles.tile([128, NET, D], BF16)
            wo_sb = singles.tile([128, G, E], BF16)
            mask_sb = singles.tile([128, 128], F32)
            ones_bf = singles.tile([128, 128], BF16)
            ones_f = singles.tile([128, 128], F32)
            qT = singles.tile([128, G, S], BF16)
            kT = singles.tile([128, S], BF16)
            v_sb = singles.tile([128, NKT, D], BF16)
            onrm = singles.tile([128, G, S], BF16)

            nc.vector.memset(ones_f, 1.0)
            nc.scalar.activation(out=ones_bf[:], in_=ones_f[:], func=AF.Copy)
            nc.sync.dma_start(out=wk_sb[:], in_=tiled_ap(wk, D, NET, D))
            nc.sync.dma_start(out=wv_sb[:], in_=tiled_ap(wv, D, NET, D))
            nc.sync.dma_start(out=wq_sb[:], in_=tiled_ap(wq, GD, NET, GD))
            nc.sync.dma_start(out=wo_sb[:], in_=tiled_ap(wo, E, G, E))
            nc.sync.dma_start(out=mask_sb[:], in_=msk[:, :])

            # ---- phase 1: projections, per s-chunk: K, V, Q ----
            for c in range(NSC):
                csl = slice(c * SC, (c + 1) * SC)
                xk_t = []
                xv_t = []
                xq_t = []
                for tg in range(4):
                    xkt = xsp.tile([128, 4, SC], BF16, tag="xk", name=f"xk{c}{tg}")
                    nc.sync.dma_start(out=xkt[:], in_=x_chunk_ap(xk, tg, c))
                    xk_t.append(xkt)
                for tg in range(4):
                    xvt = xsp.tile([128, 4, SC], BF16, tag="xv", name=f"xv{c}{tg}")
                    nc.sync.dma_start(out=xvt[:], in_=x_chunk_ap(xv, tg, c))
                    xv_t.append(xvt)
                for tg in range(4):
                    xqt = xsp.tile([128, 4, SC], BF16, tag="xq", name=f"xq{c}{tg}")
                    nc.sync.dma_start(out=xqt[:], in_=x_chunk_ap(xq, tg, c))
                    xq_t.append(xqt)

                # K^T
                ps = pacc.tile([128, SC], F32, tag="acc", name="psk")
                for t in range(NET):
                    nc.tensor.matmul(
                        ps[:],
                        lhsT=wk_sb[:, t, :],
                        rhs=xk_t[t // 4][:, t % 4, :],
                        start=(t == 0),
                        stop=(t == NET - 1),
                    )
                nc.vector.tensor_copy(out=kT[:, csl], in_=ps[:])
                # V^T -> DMA xbar transpose to V
                ps = pacc.tile([128, SC], F32, tag="acc", name="psv")
                for t in range(NET):
                    nc.tensor.matmul(
                        ps[:],
                        lhsT=wv_sb[:, t, :],
                        rhs=xv_t[t // 4][:, t % 4, :],
                        start=(t == 0),
                        stop=(t == NET - 1),
                    )
                vt_tmp = vtp.tile([128, SC], BF16, tag="vt", name="vt")
                nc.scalar.activation(out=vt_tmp[:], in_=ps[:], func=AF.Copy)
                for i in range(SC // 128):
                    nc.sync.dma_start_transpose(
                        out=v_sb[:, c * 4 + i, :],
                        in_=vt_tmp[:, i * 128 : (i + 1) * 128],
                    )
                # Q^T, 4 heads
                for h in range(G):
                    ps = pacc.tile([128, SC], F32, tag="acc", name="psq")
                    for t in range(NET):
                        nc.tensor.matmul(
                            ps[:],
                            lhsT=wq_sb[:, t, h * D : (h + 1) * D],
                            rhs=xq_t[t // 4][:, t % 4, :],
                            start=(t == 0),
                            stop=(t == NET - 1),
                        )
                    nc.scalar.activation(out=qT[:, h, csl], in_=ps[:], func=AF.Copy)

            # ---- phase 2: attention + out-projection per q-chunk ----
            for qc in range(NSC):
                qbase = qc * SC
                nkt = 4 * qc + 4  # causal k-tiles 0..nkt-1 (ascending)
                for pr in range(2):
                    l_pack = lps.tile([128, SC], F32, tag="l", name="lpack")
                    nc.vector.memset(l_pack, 0.0)
                    o_ps = [
                        ops.tile([128, SC], F32, tag="o", name=f"ops{ci}")
                        for ci in range(2)
                    ]
                    for kt in range(nkt):
                        j = kt - 4 * qc
                        qoff = 128 * j if j >= 0 else 0
                        first = kt == 0
                        last = kt == nkt - 1
                        p_t = []
                        for ci in range(2):
                            h = 2 * pr + ci
                            s_ps = sacc.tile([128, SC], F32, tag="s", name="sps")
                            nc.tensor.matmul(
                                s_ps[:, qoff:],
                                lhsT=kT[:, kt * 128 : (kt + 1) * 128],
                                rhs=qT[:, h, qbase + qoff : qbase + SC],
                                start=True,
                                stop=True,
                            )
                            if j >= 0:
                                nc.vector.tensor_add(
                                    s_ps[:, qoff : qoff + 128],
                                    s_ps[:, qoff : qoff + 128],
                                    mask_sb[:],
                                )
                            p = pexp.tile([128, SC], BF16, tag="p", name="p")
                            nc.scalar.activation(
                                out=p[:, qoff:], in_=s_ps[:, qoff:], func=AF.Exp
                            )
                            p_t.append(p)
                        for ci in range(2):
                            nc.tensor.matmul(
                                o_ps[ci][:, qoff:],
                                lhsT=v_sb[:, kt, :],
                                rhs=p_t[ci][:, qoff:],
                                start=first,
                                stop=last,
                                skip_group_check=True,
                            )
                        # two concurrent col-tiled M=1 row-sum matmuls
                        for ci in range(2):
                            nc.tensor.matmul(
                                l_pack[32 * ci : 32 * ci + 1, qoff:],
                                lhsT=ones_bf[:, 0:1],
                                rhs=p_t[ci][:, qoff:],
                                start=False,
                                stop=last,
                                skip_group_check=True,
                            )
                    # normalize: r = 1/l over partitions 0..32 in one op (rows
                    # 0 and 32 are the two heads; others are garbage), then
                    # per-head PE broadcast with base-partition-aligned APs.
                    rp_f = rpp.tile([33, SC], F32, tag="rpf", name="rpf")
                    nc.vector.reciprocal_approx_fast(
                        out=rp_f[:], in_=l_pack[0:33, :]
                    )
                    rp_bf = rpp.tile([33, SC], BF16, tag="rpb", name="rpb")
                    nc.vector.tensor_copy(out=rp_bf[:], in_=rp_f[:])
                    for ci in range(2):
                        h = 2 * pr + ci
                        rbc = pacc.tile([128, SC], F32, tag="acc", name="rbc")
                        nc.tensor.matmul(
                            rbc[:],
                            lhsT=ones_bf[32 * ci : 32 * ci + 1, :],
                            rhs=rp_bf[32 * ci : 32 * ci + 1, :],
                            start=True,
                            stop=True,
                        )
                        rbc_s = rbp.tile([128, SC], F32, tag="rbs", name="rbs")
                        nc.vector.tensor_copy(out=rbc_s[:], in_=rbc[:])
                        nc.vector.tensor_mul(
                            onrm[:, h, qbase : qbase + SC], o_ps[ci][:], rbc_s[:]
                        )

                # out-projection for this chunk's 4 s-tiles
                for sti in range(SC // 128):
                    st = qc * 4 + sti
                    stl = slice(st * 128, (st + 1) * 128)
                    ob = obp.tile([128, E], BF16, tag="ob", name="ob")
                    for ecp in range(2):
                        pse = [
                            pacc.tile([128, SC], F32, tag="acc", name=f"pso{e}")
                            for e in range(2)
                        ]
                        for h in range(G):
                            for e in range(2):
                                ec = 2 * ecp + e
                                nc.tensor.matmul(
                                    pse[e][:],
                                    lhsT=onrm[:, h, stl],
                                    rhs=wo_sb[:, h, ec * SC : (ec + 1) * SC],
                                    start=(h == 0),
                                    stop=(h == G - 1),
                                )
                        e0 = 2 * ecp * SC
                        nc.scalar.activation(
                            out=ob[:, e0 : e0 + SC], in_=pse[0][:], func=AF.Copy
                        )
                        nc.vector.tensor_copy(
                            out=ob[:, e0 + SC : e0 + 2 * SC], in_=pse[1][:]
                        )
                    nc.sync.dma_start(out=out[stl, :], in_=ob[:])
    nc.compile()
    return nc


_NC_CACHE = None


def _get_nc():
    global _NC_CACHE
    if _NC_CACHE is None:
        _NC_CACHE = build_nc()
    return _NC_CACHE


def _prep_in_maps(query, key, value, attn_mask, Wq, Wk, Wv, Wo):
    query = np.asarray(query, dtype=np.float32)
    key = np.asarray(key, dtype=np.float32)
    value = np.asarray(value, dtype=np.float32)
    Wq = np.asarray(Wq, dtype=np.float32)
    Wk = np.asarray(Wk, dtype=np.float32)
    Wv = np.asarray(Wv, dtype=np.float32)
    Wo = np.asarray(Wo, dtype=np.float32)
    am = np.asarray(attn_mask)

    xqT = [np.ascontiguousarray(query[b].T).astype(NPBF) for b in range(B)]
    xkT = [np.ascontiguousarray(key[b].T).astype(NPBF) for b in range(B)]
    xvT = [np.ascontiguousarray(value[b].T).astype(NPBF) for b in range(B)]

    # single 128x128 additive mask for the true diagonal block, [k, q] layout
    m0 = np.asarray(am[0, 0, :128, :128], dtype=np.float32)  # [q, k]
    msk_np = np.ascontiguousarray((m0.T - 1.0) * 1e9)

    in_maps = []
    for b in range(B):
        for g in range(HKV):
            in_maps.append(
                {
                    "xq": xqT[b],
                    "xk": xkT[b],
                    "xv": xvT[b],
                    "wq": np.ascontiguousarray(
                        Wq[g * GD : (g + 1) * GD, :].T * SCALE
                    ).astype(NPBF),
                    "wk": np.ascontiguousarray(
                        Wk[g * D : (g + 1) * D, :].T
                    ).astype(NPBF),
                    "wv": np.ascontiguousarray(
                        Wv[g * D : (g + 1) * D, :].T
                    ).astype(NPBF),
                    "wo": np.ascontiguousarray(
                        Wo[:, g * GD : (g + 1) * GD].T
                    ).astype(NPBF),
                    "msk": msk_np,
                }
            )
    return in_maps


def _run(inputs, trace=False, **kw):
    nc = _get_nc()
    in_maps = _prep_in_maps(**inputs)
    res = run_bass_kernel_spmd(
        nc, in_maps, list(range(NCORES)), trace=trace, **kw
    )
    outs = [np.asarray(r["out"]) for r in res.results]
    full = np.empty((B, S, E), dtype=np.float32)
    for b in range(B):
        acc = outs[b * HKV].astype(np.float32)
        for g in range(1, HKV):
            acc = acc + outs[b * HKV + g].astype(np.float32)
        full[b] = acc
    return full, res


def kernel(**inputs):
    full, _ = _run(inputs, trace=False)
    return full


# revision 4
# speedup vs baseline: 1.4222x; 1.2725x over previous
"""GQA kernel for Trainium2, 8 NeuronCores.

Sharding: core c = b*4 + g  handles batch b, kv-head g (4 query heads).
Each core computes (bf16 matmuls, fp32 PSUM):
  Q_g^T = Wq_g @ x_q^T   [4 heads][128, S]  (1/sqrt(D) folded into Wq host-side)
  K_g^T = Wk_g @ x_k^T   [128, S]
  V_g   = via V^T then DMA-xbar transpose   [S, D]
  per q-chunk (512) ascending k-tiles with causal diagonal subranges:
    S^T = K_kt @ Q^T  (only q >= k columns on diagonal tiles)
    mask add only on the single 128x128 triangular diagonal block
    p = exp(S^T) -> PV accumulate (o^T), l row-sums via col-tiled M=1
    matmuls into partitions 0/32 of one PSUM bank
  o_norm^T = o^T * bcast(1/l)   (recip_approx_fast on [33,512], PE bcast)
  out_partial = o_norm @ Wo_g.T stored bf16; host sums 4 partials/batch.

Program interleaves proj(c) -> attention(c) -> outproj(c-1) so the PE always
has ready matmul work while the scalar engine grinds exps (keeps the HAM
clock-gate warm; PE idle windows re-throttle the clock to half rate).
"""

import sys

import numpy as np

for _p in ("/opt/trn_rl_repo",):
    if _p not in sys.path:
        sys.path.insert(0, _p)

import ml_dtypes

import concourse.bass as bass
import concourse.mybir as mybir
from concourse import bacc
from concourse.bass_utils import run_bass_kernel_spmd
from concourse.tile import TileContext

B, S, E = 2, 2048, 2048
H, HKV = 16, 4
D = E // H  # 128
G = H // HKV  # 4 query heads per kv head
GD = G * D  # 512
NCORES = B * HKV  # 8
SC = 512  # q-chunk width
NSC = S // SC  # 4
NET = E // 128  # 16 e-tiles (contraction)
NKT = S // 128  # 16 k-tiles
SCALE = 1.0 / float(np.sqrt(D))

F32 = mybir.dt.float32
BF16 = mybir.dt.bfloat16
AF = mybir.ActivationFunctionType
NPBF = np.dtype(ml_dtypes.bfloat16)


def build_nc():
    nc = bacc.Bacc()
    xq = nc.declare_dram_parameter("xq", [E, S], BF16, isOutput=False)  # query[b].T
    xk = nc.declare_dram_parameter("xk", [E, S], BF16, isOutput=False)  # key[b].T
    xv = nc.declare_dram_parameter("xv", [E, S], BF16, isOutput=False)  # value[b].T
    wq = nc.declare_dram_parameter("wq", [E, GD], BF16, isOutput=False)
    wk = nc.declare_dram_parameter("wk", [E, D], BF16, isOutput=False)
    wv = nc.declare_dram_parameter("wv", [E, D], BF16, isOutput=False)
    wo = nc.declare_dram_parameter("wo", [GD, E], BF16, isOutput=False)
    msk = nc.declare_dram_parameter("msk", [128, 128], F32, isOutput=False)
    out = nc.declare_dram_parameter("out", [S, E], BF16, isOutput=True)

    def tiled_ap(dram, rowstride, ntile, ncol):
        # [128, ntile, ncol] view of a row-major DRAM [ntile*128, ncol] tensor
        base = dram[:, :]
        return bass.AP(
            tensor=base.tensor,
            offset=base.offset,
            ap=[[rowstride, 128], [128 * rowstride, ntile], [1, ncol]],
        )

    def x_chunk_ap(dram, tg, c):
        # e-tiles 4*tg..4*tg+3, s-columns [c*SC, (c+1)*SC)
        base = dram[:, :]
        return bass.AP(
            tensor=base.tensor,
            offset=base.offset + tg * 4 * 128 * S + c * SC,
            ap=[[S, 128], [128 * S, 4], [1, SC]],
        )

    with TileContext(nc) as tc:
        with (
            tc.tile_pool(name="singles", bufs=1) as singles,
            tc.tile_pool(name="xs", bufs=6) as xsp,
            tc.tile_pool(name="pexp", bufs=4) as pexp,
            tc.tile_pool(name="vtp", bufs=2) as vtp,
            tc.tile_pool(name="rpp", bufs=2) as rpp,
            tc.tile_pool(name="rbp", bufs=2) as rbp,
            tc.tile_pool(name="obp", bufs=2) as obp,
            tc.tile_pool(name="pacc", bufs=2, space="PSUM") as pacc,
            tc.tile_pool(name="sacc", bufs=3, space="PSUM") as sacc,
            tc.tile_pool(name="ops", bufs=2, space="PSUM") as ops,
            tc.tile_pool(name="lps", bufs=1, space="PSUM") as lps,
        ):
            # ---- resident weights / constants ----
            wq_sb = singles.tile([128, NET, GD], BF16)
            wk_sb = singles.tile([128, NET, D], BF16)
            wv_sb = singles.tile([128, NET, D], BF16)
            wo_sb = singles.tile([128, G, E], BF16)
            mask_sb = singles.tile([128, 128], F32)
            ones_bf = singles.tile([128, 128], BF16)
            ones_f = singles.tile([128, 128], F32)
            qT = singles.tile([128, G, S], BF16)
            kT = singles.tile([128, S], BF16)
            v_sb = singles.tile([128, NKT, D], BF16)
            onrm = singles.tile([128, G, S], BF16)

            nc.vector.memset(ones_f, 1.0)
            nc.scalar.activation(out=ones_bf[:], in_=ones_f[:], func=AF.Copy)
            nc.sync.dma_start(out=wk_sb[:], in_=tiled_ap(wk, D, NET, D))
            nc.sync.dma_start(out=wv_sb[:], in_=tiled_ap(wv, D, NET, D))
            nc.sync.dma_start(out=wq_sb[:], in_=tiled_ap(wq, GD, NET, GD))
            nc.sync.dma_start(out=wo_sb[:], in_=tiled_ap(wo, E, G, E))
            nc.sync.dma_start(out=mask_sb[:], in_=msk[:, :])

            def proj(c):
                csl = slice(c * SC, (c + 1) * SC)
                xk_t, xv_t, xq_t = [], [], []
                for tg in range(4):
                    xkt = xsp.tile([128, 4, SC], BF16, tag="xk", name=f"xk{c}{tg}")
                    nc.sync.dma_start(out=xkt[:], in_=x_chunk_ap(xk, tg, c))
                    xk_t.append(xkt)
                for tg in range(4):
                    xvt = xsp.tile([128, 4, SC], BF16, tag="xv", name=f"xv{c}{tg}")
                    nc.sync.dma_start(out=xvt[:], in_=x_chunk_ap(xv, tg, c))
                    xv_t.append(xvt)
                for tg in range(4):
                    xqt = xsp.tile([128, 4, SC], BF16, tag="xq", name=f"xq{c}{tg}")
                    nc.sync.dma_start(out=xqt[:], in_=x_chunk_ap(xq, tg, c))
                    xq_t.append(xqt)
                # K^T
                ps = pacc.tile([128, SC], F32, tag="acc", name="psk")
                for t in range(NET):
                    nc.tensor.matmul(
                        ps[:],
                        lhsT=wk_sb[:, t, :],
                        rhs=xk_t[t // 4][:, t % 4, :],
                        start=(t == 0),
                        stop=(t == NET - 1),
                    )
                nc.vector.tensor_copy(out=kT[:, csl], in_=ps[:])
                # V^T -> DMA xbar transpose to V
                ps = pacc.tile([128, SC], F32, tag="acc", name="psv")
                for t in range(NET):
                    nc.tensor.matmul(
                        ps[:],
                        lhsT=wv_sb[:, t, :],
                        rhs=xv_t[t // 4][:, t % 4, :],
                        start=(t == 0),
                        stop=(t == NET - 1),
                    )
                vt_tmp = vtp.tile([128, SC], BF16, tag="vt", name="vt")
                nc.scalar.activation(out=vt_tmp[:], in_=ps[:], func=AF.Copy)
                for i in range(SC // 128):
                    nc.sync.dma_start_transpose(
                        out=v_sb[:, c * 4 + i, :],
                        in_=vt_tmp[:, i * 128 : (i + 1) * 128],
                    )
                # Q^T, 4 heads
                for h in range(G):
                    ps = pacc.tile([128, SC], F32, tag="acc", name="psq")
                    for t in range(NET):
                        nc.tensor.matmul(
                            ps[:],
                            lhsT=wq_sb[:, t, h * D : (h + 1) * D],
                            rhs=xq_t[t // 4][:, t % 4, :],
                            start=(t == 0),
                            stop=(t == NET - 1),
                        )
                    nc.scalar.activation(out=qT[:, h, csl], in_=ps[:], func=AF.Copy)

            def attention(qc):
                qbase = qc * SC
                nkt = 4 * qc + 4  # causal k-tiles 0..nkt-1 (ascending)
                for pr in range(2):
                    l_pack = lps.tile([128, SC], F32, tag="l", name="lpack")
                    nc.vector.memset(l_pack, 0.0)
                    o_ps = [
                        ops.tile([128, SC], F32, tag="o", name=f"ops{ci}")
                        for ci in range(2)
                    ]
                    for kt in range(nkt):
                        j = kt - 4 * qc
                        qoff = 128 * j if j >= 0 else 0
                        first = kt == 0
                        last = kt == nkt - 1
                        p_t = []
                        for ci in range(2):
                            h = 2 * pr + ci
                            s_ps = sacc.tile([128, SC], F32, tag="s", name="sps")
                            nc.tensor.matmul(
                                s_ps[:, qoff:],
                                lhsT=kT[:, kt * 128 : (kt + 1) * 128],
                                rhs=qT[:, h, qbase + qoff : qbase + SC],
                                start=True,
                                stop=True,
                            )
                            if j >= 0:
                                nc.vector.tensor_add(
                                    s_ps[:, qoff : qoff + 128],
                                    s_ps[:, qoff : qoff + 128],
                                    mask_sb[:],
                                )
                            p = pexp.tile([128, SC], BF16, tag="p", name="p")
                            nc.scalar.activation(
                                out=p[:, qoff:], in_=s_ps[:, qoff:], func=AF.Exp
                            )
                            p_t.append(p)
                        for ci in range(2):
                            nc.tensor.matmul(
                                o_ps[ci][:, qoff:],
                                lhsT=v_sb[:, kt, :],
                                rhs=p_t[ci][:, qoff:],
                                start=first,
                                stop=last,
                                skip_group_check=True,
                            )
                        for ci in range(2):
                            nc.tensor.matmul(
                                l_pack[32 * ci : 32 * ci + 1, qoff:],
                                lhsT=ones_bf[:, 0:1],
                                rhs=p_t[ci][:, qoff:],
                                start=False,
                                stop=last,
                                skip_group_check=True,
                            )
                    # normalize: r = 1/l over partitions 0..32 in one DVE op
                    rp_f = rpp.tile([33, SC], F32, tag="rpf", name="rpf")
                    nc.vector.reciprocal_approx_fast(
                        out=rp_f[:], in_=l_pack[0:33, :]
                    )
                    rp_bf = rpp.tile([33, SC], BF16, tag="rpb", name="rpb")
                    nc.vector.tensor_copy(out=rp_bf[:], in_=rp_f[:])
                    for ci in range(2):
                        h = 2 * pr + ci
                        rbc = pacc.tile([128, SC], F32, tag="acc", name="rbc")
                        nc.tensor.matmul(
                            rbc[:],
                            lhsT=ones_bf[32 * ci : 32 * ci + 1, :],
                            rhs=rp_bf[32 * ci : 32 * ci + 1, :],
                            start=True,
                            stop=True,
                        )
                        rbc_s = rbp.tile([128, SC], F32, tag="rbs", name="rbs")
                        nc.vector.tensor_copy(out=rbc_s[:], in_=rbc[:])
                        nc.vector.tensor_mul(
                            onrm[:, h, qbase : qbase + SC], o_ps[ci][:], rbc_s[:]
                        )

            def outproj(qc):
                for sti in range(SC // 128):
                    st = qc * 4 + sti
                    stl = slice(st * 128, (st + 1) * 128)
                    ob = obp.tile([128, E], BF16, tag="ob", name="ob")
                    for ecp in range(2):
                        pse = [
                            pacc.tile([128, SC], F32, tag="acc", name=f"pso{e}")
                            for e in range(2)
                        ]
                        for h in range(G):
                            for e in range(2):
                                ec = 2 * ecp + e
                                nc.tensor.matmul(
                                    pse[e][:],
                                    lhsT=onrm[:, h, stl],
                                    rhs=wo_sb[:, h, ec * SC : (ec + 1) * SC],
                                    start=(h == 0),
                                    stop=(h == G - 1),
                                )
                        e0 = 2 * ecp * SC
                        nc.scalar.activation(
                            out=ob[:, e0 : e0 + SC], in_=pse[0][:], func=AF.Copy
                        )
                        nc.vector.tensor_copy(
                            out=ob[:, e0 + SC : e0 + 2 * SC], in_=pse[1][:]
                        )
                    nc.sync.dma_start(out=out[stl, :], in_=ob[:])

            # proj(c) feeds attention(c); outproj(c) emitted one segment later
            # so its PE-dense matmuls fill the exp-bound attention stretches.
            proj(0)
            attention(0)
            for c in range(1, NSC):
                proj(c)
                attention(c)
                outproj(c - 1)
            outproj(NSC - 1)
    nc.compile()
    return nc


_NC_CACHE = None


def _get_nc():
    global _NC_CACHE
    if _NC_CACHE is None:
        _NC_CACHE = build_nc()
    return _NC_CACHE


def _prep_in_maps(query, key, value, attn_mask, Wq, Wk, Wv, Wo):
    query = np.asarray(query, dtype=np.float32)
    key = np.asarray(key, dtype=np.float32)
    value = np.asarray(value, dtype=np.float32)
    Wq = np.asarray(Wq, dtype=np.float32)
    Wk = np.asarray(Wk, dtype=np.float32)
    Wv = np.asarray(Wv, dtype=np.float32)
    Wo = np.asarray(Wo, dtype=np.float32)
    am = np.asarray(attn_mask)

    xqT = [np.ascontiguousarray(query[b].T).astype(NPBF) for b in range(B)]
    xkT = [np.ascontiguousarray(key[b].T).astype(NPBF) for b in range(B)]
    xvT = [np.ascontiguousarray(value[b].T).astype(NPBF) for b in range(B)]

    # single 128x128 additive mask for the true diagonal block, [k, q] layout
    m0 = np.asarray(am[0, 0, :128, :128], dtype=np.float32)  # [q, k]
    msk_np = np.ascontiguousarray((m0.T - 1.0) * 1e9)

    in_maps = []
    for b in range(B):
        for g in range(HKV):
            in_maps.append(
                {
                    "xq": xqT[b],
                    "xk": xkT[b],
                    "xv": xvT[b],
                    "wq": np.ascontiguousarray(
                        Wq[g * GD : (g + 1) * GD, :].T * SCALE
                    ).astype(NPBF),
                    "wk": np.ascontiguousarray(
                        Wk[g * D : (g + 1) * D, :].T
                    ).astype(NPBF),
                    "wv": np.ascontiguousarray(
                        Wv[g * D : (g + 1) * D, :].T
                    ).astype(NPBF),
                    "wo": np.ascontiguousarray(
                        Wo[:, g * GD : (g + 1) * GD].T
                    ).astype(NPBF),
                    "msk": msk_np,
                }
            )
    return in_maps


def _run(inputs, trace=False, **kw):
    nc = _get_nc()
    in_maps = _prep_in_maps(**inputs)
    res = run_bass_kernel_spmd(
        nc, in_maps, list(range(NCORES)), trace=trace, **kw
    )
    outs = [np.asarray(r["out"]) for r in res.results]
    full = np.empty((B, S, E), dtype=np.float32)
    for b in range(B):
        acc = outs[b * HKV].astype(np.float32)
        for g in range(1, HKV):
            acc = acc + outs[b * HKV + g].astype(np.float32)
        full[b] = acc
    return full, res


def kernel(**inputs):
    full, _ = _run(inputs, trace=False)
    return full


# revision 7
# speedup vs baseline: 1.4272x; 1.0035x over previous
"""GQA kernel for Trainium2, 8 NeuronCores.

Sharding: core c = b*4 + g  handles batch b, kv-head g (4 query heads).
Each core computes (bf16 matmuls, fp32 PSUM):
  Q_g^T = Wq_g @ x_q^T   [4 heads][128, S]  (1/sqrt(D) folded into Wq host-side)
  K_g^T = Wk_g @ x_k^T   [128, S]
  V_g   = via V^T then DMA-xbar transpose   [S, D]
  per q-chunk (512), per head, ascending k-tile PAIRS with causal diagonal
  subranges ([128,1024] score tiles -> one exp per pair):
    S^T = K_kt @ Q^T  (only q >= k columns on diagonal tiles)
    mask add only on the single 128x128 triangular diagonal block
    p = exp(S^T) kept in SBUF (p_all) -> PV accumulates o^T
  l row-sums = ones^T @ p as a head-end matmul burst (PSUM bank borrowed
  from the shared accumulator pool), r = recip_approx_fast(l), broadcast
  across partitions with a stride-0 DMA, o_norm^T = o^T * r.
  out_partial = o_norm @ Wo_g.T stored bf16; host sums 4 partials/batch.

Program interleaves x-chunk DMA loads, projections, attention heads and
out-projection s-tiles so the PE always has ready matmul work while the
scalar engine grinds exps (PE idle windows re-throttle the HAM clock gate
to half rate, which is the main perf cliff on trn2).
"""

import sys

import numpy as np

for _p in ("/opt/trn_rl_repo",):
    if _p not in sys.path:
        sys.path.insert(0, _p)

import ml_dtypes

import concourse.bass as bass
import concourse.mybir as mybir
from concourse import bacc
from concourse.bass_utils import run_bass_kernel_spmd
from concourse.tile import TileContext

B, S, E = 2, 2048, 2048
H, HKV = 16, 4
D = E // H  # 128
G = H // HKV  # 4 query heads per kv head
GD = G * D  # 512
NCORES = B * HKV  # 8
SC = 512  # q-chunk width
NSC = S // SC  # 4
NET = E // 128  # 16 e-tiles (contraction)
NKT = S // 128  # 16 k-tiles
SCALE = 1.0 / float(np.sqrt(D))

F32 = mybir.dt.float32
BF16 = mybir.dt.bfloat16
AF = mybir.ActivationFunctionType
NPBF = np.dtype(ml_dtypes.bfloat16)


def build_nc():
    nc = bacc.Bacc()
    xq = nc.declare_dram_parameter("xq", [E, S], BF16, isOutput=False)  # query[b].T
    xk = nc.declare_dram_parameter("xk", [E, S], BF16, isOutput=False)  # key[b].T
    xv = nc.declare_dram_parameter("xv", [E, S], BF16, isOutput=False)  # value[b].T
    wq = nc.declare_dram_parameter("wq", [E, GD], BF16, isOutput=False)
    wk = nc.declare_dram_parameter("wk", [E, D], BF16, isOutput=False)
    wv = nc.declare_dram_parameter("wv", [E, D], BF16, isOutput=False)
    wo = nc.declare_dram_parameter("wo", [GD, E], BF16, isOutput=False)
    msk = nc.declare_dram_parameter("msk", [128, 128], F32, isOutput=False)
    out = nc.declare_dram_parameter("out", [S, E], BF16, isOutput=True)

    def tiled_ap(dram, rowstride, ntile, ncol):
        # [128, ntile, ncol] view of a row-major DRAM [ntile*128, ncol] tensor
        base = dram[:, :]
        return bass.AP(
            tensor=base.tensor,
            offset=base.offset,
            ap=[[rowstride, 128], [128 * rowstride, ntile], [1, ncol]],
        )

    def x_chunk_ap(dram, tg, c):
        # e-tiles 4*tg..4*tg+3, s-columns [c*SC, (c+1)*SC)
        base = dram[:, :]
        return bass.AP(
            tensor=base.tensor,
            offset=base.offset + tg * 4 * 128 * S + c * SC,
            ap=[[S, 128], [128 * S, 4], [1, SC]],
        )

    with TileContext(nc) as tc:
        with (
            tc.tile_pool(name="singles", bufs=1) as singles,
            tc.tile_pool(name="xs", bufs=6) as xsp,
            tc.tile_pool(name="vtp", bufs=2) as vtp,
            tc.tile_pool(name="rpp", bufs=2) as rpp,
            tc.tile_pool(name="rbp", bufs=2) as rbp,
            tc.tile_pool(name="obp", bufs=2) as obp,
            tc.tile_pool(name="drp", bufs=2, space="DRAM") as drp,
            tc.tile_pool(name="pacc", bufs=2, space="PSUM") as pacc,
            tc.tile_pool(name="sacc", bufs=2, space="PSUM") as sacc,
            tc.tile_pool(name="ops", bufs=2, space="PSUM") as ops,
        ):
            # ---- resident weights / constants ----
            wq_sb = singles.tile([128, NET, GD], BF16)
            wk_sb = singles.tile([128, NET, D], BF16)
            wv_sb = singles.tile([128, NET, D], BF16)
            wo_sb = singles.tile([128, G, E], BF16)
            mask_sb = singles.tile([128, 128], F32)
            ones_bf = singles.tile([128, 128], BF16)
            ones_f = singles.tile([128, 128], F32)
            qT = singles.tile([128, G, S], BF16)
            kT = singles.tile([128, S], BF16)
            v_sb = singles.tile([128, NKT, D], BF16)
            onrm = singles.tile([128, G, 2, SC], BF16)  # 2-chunk rotation
            p_all = singles.tile([128, NKT * SC], BF16)

            nc.vector.memset(ones_f, 1.0)
            nc.scalar.activation(out=ones_bf[:], in_=ones_f[:], func=AF.Copy)
            nc.sync.dma_start(out=wk_sb[:], in_=tiled_ap(wk, D, NET, D))
            nc.sync.dma_start(out=wv_sb[:], in_=tiled_ap(wv, D, NET, D))
            nc.sync.dma_start(out=wq_sb[:], in_=tiled_ap(wq, GD, NET, GD))
            nc.sync.dma_start(out=wo_sb[:], in_=tiled_ap(wo, E, G, E))
            nc.sync.dma_start(out=mask_sb[:], in_=msk[:, :])

            def load(c):
                tiles = {"xk": [], "xv": [], "xq": []}
                for nm, dram in (("xk", xk), ("xv", xv), ("xq", xq)):
                    for tg in range(4):
                        t = xsp.tile(
                            [128, 4, SC], BF16, tag=nm, name=f"{nm}{c}{tg}"
                        )
                        nc.sync.dma_start(out=t[:], in_=x_chunk_ap(dram, tg, c))
                        tiles[nm].append(t)
                return tiles

            def proj(c, tiles):
                csl = slice(c * SC, (c + 1) * SC)
                xk_t, xv_t, xq_t = tiles["xk"], tiles["xv"], tiles["xq"]
                # K^T
                ps = pacc.tile([128, SC], F32, tag="acc", name="psk")
                for t in range(NET):
                    nc.tensor.matmul(
                        ps[:],
                        lhsT=wk_sb[:, t, :],
                        rhs=xk_t[t // 4][:, t % 4, :],
                        start=(t == 0),
                        stop=(t == NET - 1),
                    )
                nc.vector.tensor_copy(out=kT[:, csl], in_=ps[:])
                # V^T -> DMA xbar transpose to V
                ps = pacc.tile([128, SC], F32, tag="acc", name="psv")
                for t in range(NET):
                    nc.tensor.matmul(
                        ps[:],
                        lhsT=wv_sb[:, t, :],
                        rhs=xv_t[t // 4][:, t % 4, :],
                        start=(t == 0),
                        stop=(t == NET - 1),
                    )
                vt_tmp = vtp.tile([128, SC], BF16, tag="vt", name="vt")
                nc.scalar.activation(out=vt_tmp[:], in_=ps[:], func=AF.Copy)
                for i in range(SC // 128):
                    nc.sync.dma_start_transpose(
                        out=v_sb[:, c * 4 + i, :],
                        in_=vt_tmp[:, i * 128 : (i + 1) * 128],
                    )
                # Q^T, 4 heads
                for h in range(G):
                    ps = pacc.tile([128, SC], F32, tag="acc", name="psq")
                    for t in range(NET):
                        nc.tensor.matmul(
                            ps[:],
                            lhsT=wq_sb[:, t, h * D : (h + 1) * D],
                            rhs=xq_t[t // 4][:, t % 4, :],
                            start=(t == 0),
                            stop=(t == NET - 1),
                        )
                    nc.scalar.activation(out=qT[:, h, csl], in_=ps[:], func=AF.Copy)

            def attn_head(qc, h):
                qbase = qc * SC
                nkt = 4 * qc + 4  # causal k-tiles, ascending, in pairs
                o_ps = ops.tile([128, SC], F32, tag="o", name="ops")
                for kp in range(nkt // 2):
                    kts = (2 * kp, 2 * kp + 1)
                    s2 = sacc.tile([128, 2 * SC], F32, tag="s", name="sps")
                    offs = []
                    for i, kt in enumerate(kts):
                        j = kt - 4 * qc
                        qoff = 128 * j if j >= 0 else 0
                        offs.append(qoff)
                        nc.tensor.matmul(
                            s2[:, i * SC + qoff : (i + 1) * SC],
                            lhsT=kT[:, kt * 128 : (kt + 1) * 128],
                            rhs=qT[:, h, qbase + qoff : qbase + SC],
                            start=True,
                            stop=True,
                        )
                        if j >= 0:
                            nc.vector.tensor_add(
                                s2[:, i * SC + qoff : i * SC + qoff + 128],
                                s2[:, i * SC + qoff : i * SC + qoff + 128],
                                mask_sb[:],
                            )
                    diag = kts[1] >= 4 * qc
                    if diag:
                        for i, kt in enumerate(kts):
                            nc.scalar.activation(
                                out=p_all[:, kt * SC + offs[i] : (kt + 1) * SC],
                                in_=s2[:, i * SC + offs[i] : (i + 1) * SC],
                                func=AF.Exp,
                            )
                    else:
                        nc.scalar.activation(
                            out=p_all[:, kts[0] * SC : (kts[1] + 1) * SC],
                            in_=s2[:],
                            func=AF.Exp,
                        )
                    for i, kt in enumerate(kts):
                        nc.tensor.matmul(
                            o_ps[:, offs[i] :],
                            lhsT=v_sb[:, kt, :],
                            rhs=p_all[:, kt * SC + offs[i] : (kt + 1) * SC],
                            start=(kt == 0),
                            stop=(kt == nkt - 1),
                            skip_group_check=True,
                        )
                # head-end l burst: l = ones^T @ p over all k-tiles
                l_acc = pacc.tile([128, SC], F32, tag="acc", name="lacc")
                for kt in range(nkt):
                    j = kt - 4 * qc
                    qoff = 128 * j if j >= 0 else 0
                    nc.tensor.matmul(
                        l_acc[0:1, qoff:],
                        lhsT=ones_bf[:, 0:1],
                        rhs=p_all[:, kt * SC + qoff : (kt + 1) * SC],
                        start=(kt == 0),
                        stop=(kt == nkt - 1),
                        skip_group_check=True,
                    )
                rp_f = rpp.tile([1, SC], F32, tag="rpf", name="rpf")
                nc.vector.reciprocal_approx_fast(out=rp_f[:], in_=l_acc[0:1, :])
                # broadcast r across partitions: DRAM bounce + stride-0 read
                rp_d = drp.tile([1, SC], F32, tag="rpd", name="rpd")
                nc.sync.dma_start(out=rp_d[:], in_=rp_f[:])
                rbc_s = rbp.tile([128, SC], F32, tag="rbs", name="rbs")
                rp_b = rp_d[:]
                nc.sync.dma_start(
                    out=rbc_s[:],
                    in_=bass.AP(
                        tensor=rp_b.tensor,
                        offset=rp_b.offset,
                        ap=[[0, 128]] + list(rp_b.ap[1:]),
                    ),
                )
                nc.vector.tensor_mul(
                    onrm[:, h, qc % 2, :], o_ps[:], rbc_s[:]
                )

            def outproj_st(qc, sti):
                st = qc * 4 + sti
                stl = slice(st * 128, (st + 1) * 128)
                lsl = slice(sti * 128, (sti + 1) * 128)
                ob = obp.tile([128, E], BF16, tag="ob", name="ob")
                for ecp in range(2):
                    pse = [
                        pacc.tile([128, SC], F32, tag="acc", name=f"pso{e}")
                        for e in range(2)
                    ]
                    for h in range(G):
                        for e in range(2):
                            ec = 2 * ecp + e
                            nc.tensor.matmul(
                                pse[e][:],
                                lhsT=onrm[:, h, qc % 2, lsl],
                                rhs=wo_sb[:, h, ec * SC : (ec + 1) * SC],
                                start=(h == 0),
                                stop=(h == G - 1),
                            )
                    e0 = 2 * ecp * SC
                    nc.scalar.activation(
                        out=ob[:, e0 : e0 + SC], in_=pse[0][:], func=AF.Copy
                    )
                    nc.vector.tensor_copy(
                        out=ob[:, e0 + SC : e0 + 2 * SC], in_=pse[1][:]
                    )
                nc.sync.dma_start(out=out[stl, :], in_=ob[:])

            # pipeline: loads run one chunk ahead; outproj(c-1) s-tiles are
            # interleaved between attention heads of chunk c.
            tiles = load(0)
            proj(0, tiles)
            tiles = load(1)
            for h in range(G):
                attn_head(0, h)
            proj(1, tiles)
            tiles = load(2)
            for h in range(G):
                attn_head(1, h)
                outproj_st(0, h)
            proj(2, tiles)
            tiles = load(3)
            for h in range(G):
                attn_head(2, h)
                outproj_st(1, h)
            proj(3, tiles)
            for h in range(G):
                attn_head(3, h)
                outproj_st(2, h)
            for sti in range(4):
                outproj_st(3, sti)
    nc.compile()
    return nc


_NC_CACHE = None


def _get_nc():
    global _NC_CACHE
    if _NC_CACHE is None:
        _NC_CACHE = build_nc()
    return _NC_CACHE


def _prep_in_maps(query, key, value, attn_mask, Wq, Wk, Wv, Wo):
    query = np.asarray(query, dtype=np.float32)
    key = np.asarray(key, dtype=np.float32)
    value = np.asarray(value, dtype=np.float32)
    Wq = np.asarray(Wq, dtype=np.float32)
    Wk = np.asarray(Wk, dtype=np.float32)
    Wv = np.asarray(Wv, dtype=np.float32)
    Wo = np.asarray(Wo, dtype=np.float32)
    am = np.asarray(attn_mask)

    xqT = [np.ascontiguousarray(query[b].T).astype(NPBF) for b in range(B)]
    xkT = [np.ascontiguousarray(key[b].T).astype(NPBF) for b in range(B)]
    xvT = [np.ascontiguousarray(value[b].T).astype(NPBF) for b in range(B)]

    # single 128x128 additive mask for the true diagonal block, [k, q] layout
    m0 = np.asarray(am[0, 0, :128, :128], dtype=np.float32)  # [q, k]
    msk_np = np.ascontiguousarray((m0.T - 1.0) * 1e9)

    in_maps = []
    for b in range(B):
        for g in range(HKV):
            in_maps.append(
                {
                    "xq": xqT[b],
                    "xk": xkT[b],
                    "xv": xvT[b],
                    "wq": np.ascontiguousarray(
                        Wq[g * GD : (g + 1) * GD, :].T * SCALE
                    ).astype(NPBF),
                    "wk": np.ascontiguousarray(
                        Wk[g * D : (g + 1) * D, :].T
                    ).astype(NPBF),
                    "wv": np.ascontiguousarray(
                        Wv[g * D : (g + 1) * D, :].T
                    ).astype(NPBF),
                    "wo": np.ascontiguousarray(
                        Wo[:, g * GD : (g + 1) * GD].T
                    ).astype(NPBF),
                    "msk": msk_np,
                }
            )
    return in_maps


def _run(inputs, trace=False, **kw):
    nc = _get_nc()
    in_maps = _prep_in_maps(**inputs)
    res = run_bass_kernel_spmd(
        nc, in_maps, list(range(NCORES)), trace=trace, **kw
    )
    outs = [np.asarray(r["out"]) for r in res.results]
    full = np.empty((B, S, E), dtype=np.float32)
    for b in range(B):
        acc = outs[b * HKV].astype(np.float32)
        for g in range(1, HKV):
            acc = acc + outs[b * HKV + g].astype(np.float32)
        full[b] = acc
    return full, res


def kernel(**inputs):
    full, _ = _run(inputs, trace=False)
    return full
